# revision 1
# baseline (speedup 1.0000x reference)
"""Int8-quantized linear: y = x @ (w_q * scale)^T + bias, tensor-parallel on 8 cores.

Shapes (hardcoded): x [4,32,4096] f32, w_q [11008,4096] int8, scale [1] f32,
bias [11008] f32 -> out [4,32,11008] f32.

Strategy: column-parallel over out_features (1376 per core). Each core streams
its int8 weight shard (pre-transposed on host to [4096,1376] so the contraction
dim lands on SBUF partitions), upconverts int8->fp16 on-chip, and runs fp16
matmuls with the scale-folded fp16 activations as the stationary operand. The
shard's columns are split 896/480 into a DVE-converted and an ACT-converted
stream (separate DRAM->SBUF DMAs and separate SBUF tiles) so every tile has
exactly one writer and one reader engine - the walrus build in this env allows
at most 2 sync-wait commands per instruction. Bias enters PSUM through a K=2
ones-matmul (fp16 hi+lo split keeps it near-fp32-exact). PSUM is evicted
through ScalarE and DMA'd out; the host concatenates the 8 feature shards.
"""

import numpy as np

P = 128            # partitions = B*S tokens
IN_F = 4096
OUT_F = 11008
N_CORES = 8
N_SHARD = OUT_F // N_CORES          # 1376
K_CHUNKS = IN_F // P                # 32
COLS_A = 960       # DVE-converted columns (per k-chunk; DVE runs 2 elem/cyc)
COLS_B = N_SHARD - COLS_A           # 416, ACT-converted (1 elem/cyc)
# (tile_key, offset within tile, size, global out offset); each <=512 fp32/bank
BANKS = [("a", 0, 512, 0), ("a", 512, COLS_A - 512, 512), ("b", 0, COLS_B, COLS_A)]
# One SBUF slot per generation for all weight tiles: slot recycling creates
# WAR/WAW sync waits, and this env's walrus allows at most 1 wait on a DMA
# instruction (2 on compute). Total SBUF stays ~156KB/partition.
# k-chunks per weight DMA (HWDGE trigger cost is per-transfer; first groups
# smaller so the first matmul isn't gated on a 700KB transfer)
WGROUPS = [1, 1, 2, 4, 4, 4, 4, 4, 4, 4]
WBUFS = len(WGROUPS)
W16BUFS = K_CHUNKS

_CACHE = {}


def _patch_tile_drain():
    """The walrus build in this env rejects >2 sync-wait commands on one
    instruction; Tile's kernel-tail drain aggregates one wait per live
    semaphore. Re-emit the tail as one single-wait drain per outstanding
    proc (semantically identical: SP serially waits each sem, then the
    usual all-engine barrier runs)."""
    import concourse.tile as tile
    from concourse.vector_clock import ScopedClock, VectorClock

    if getattr(tile.TileContext, "_ant_drain_patched", False):
        return
    N_PROCS = 27

    def _drain_and_barrier(self, tick_clock, wait_clock):
        gc = tick_clock.global_clock
        live = [p for p in range(N_PROCS) if gc[p] > 0]
        for p in live:
            vc = VectorClock([gc[q] if q == p else 0 for q in range(N_PROCS)])
            d = self.nc.sync.drain()
            wait_clock.add_sem_waits(d.ins, ScopedClock({None: vc}))
        if not live:
            self.nc.sync.drain()
        self.nc.all_engine_barrier()
        assert self.sems is not None
        popped = self.nc._tile_sem_poison_stack.pop()
        assert popped is self._sem_poison
        self.nc.clear_and_free_semaphores(list(self.sems.allocated().values()))
        self.nc.all_engine_barrier()

    tile.TileContext._drain_and_barrier = _drain_and_barrier
    tile.TileContext._ant_drain_patched = True


def _build_nc():
    import concourse.bass as bass
    import concourse.mybir as mybir
    import concourse.tile as tile

    _patch_tile_drain()
    nc = bass.Bass()
    xs = nc.declare_dram_parameter("xs", [P, IN_F], mybir.dt.float16, isOutput=False)
    wq = nc.declare_dram_parameter("wq", [IN_F, N_SHARD], mybir.dt.int8, isOutput=False)
    bi = nc.declare_dram_parameter("bias2", [2, N_SHARD], mybir.dt.float16, isOutput=False)
    out = nc.declare_dram_parameter("out", [P, N_SHARD], mybir.dt.float32, isOutput=True)

    with tile.TileContext(nc) as tc:
        with tc.tile_pool(name="const", bufs=1) as cpool, \
             tc.tile_pool(name="w8", bufs=WBUFS) as w8p, \
             tc.tile_pool(name="w16a", bufs=W16BUFS) as w16ap, \
             tc.tile_pool(name="w16b", bufs=W16BUFS) as w16bp, \
             tc.tile_pool(name="ps", bufs=1, space="PSUM") as psp, \
             tc.tile_pool(name="ob", bufs=1) as obp:
            # x loaded in 4 uneven chunks (small first), interleaved ahead of
            # the weight groups on the same (SP) trigger engine
            XKS = [2, 6, 12, 12]   # k-chunks per x tile
            NX = len(XKS)
            xko = [sum(XKS[:i]) for i in range(NX + 1)]
            xts = [cpool.tile([P, XKS[i] * P], mybir.dt.float16,
                              name=f"xq{i}", tag=f"xq{i}") for i in range(NX)]

            def xslice(k):
                i = next(i for i in range(NX) if xko[i] <= k < xko[i + 1])
                o = (k - xko[i]) * P
                return xts[i][:, o:o + P]

            ones_t = cpool.tile([2, P], mybir.dt.float16)
            nc.vector.memset(ones_t[:], 1.0)
            bias_t = cpool.tile([2, N_SHARD], mybir.dt.float16)
            nc.gpsimd.dma_start(out=bias_t[:], in_=bi[:])

            psums = [
                psp.tile([P, sz], mybir.dt.float32, name=f"psum{j}", tag=f"psum{j}")
                for j, (_, _, sz, _) in enumerate(BANKS)
            ]
            k = 0
            w8s = []
            # x chunk i is placed just before the weight group that makes it
            # needed (x on SWDGE: keeps every HWDGE queue private to one
            # weight transfer, so no cross-trigger FIFO waits)
            XPLACE = {0: 0, 1: 1, 4: 2, 6: 3}
            for g, gsz in enumerate(WGROUPS):
                if g in XPLACE:
                    i = XPLACE[g]
                    nc.gpsimd.dma_start(
                        out=xts[i][:], in_=xs[:, xko[i] * P:xko[i + 1] * P])
                # one DMA carries gsz k-chunks: SBUF col block t holds chunk
                # k+t (3D DRAM AP, contiguous SBUF rows)
                w8 = w8p.tile([P, gsz * N_SHARD], mybir.dt.int8, name=f"w8_{g}", tag="w8")
                src = wq[k * P:(k + gsz) * P, :].rearrange("(t p) n -> p t n", p=P)
                dst = w8[:].rearrange("p (t n) -> p t n", t=gsz)
                nc.sync.dma_start(out=dst, in_=src)
                w8s.append((w8, k, gsz))
                k += gsz
            k = 0
            for w8, k0, gsz in w8s:
                for t in range(gsz):
                    k = k0 + t
                    co = t * N_SHARD
                    w16a = w16ap.tile([P, COLS_A], mybir.dt.float16)
                    nc.vector.tensor_copy(out=w16a[:], in_=w8[:, co:co + COLS_A])
                    w16b = w16bp.tile([P, COLS_B], mybir.dt.float16)
                    nc.scalar.copy(out=w16b[:], in_=w8[:, co + COLS_A:co + N_SHARD])
                    wt = {"a": w16a, "b": w16b}
                    xsl = xslice(k)
                    for j, (key, to, sz, _) in enumerate(BANKS):
                        nc.tensor.matmul(
                            psums[j][:], lhsT=xsl,
                            rhs=wt[key][:, to:to + sz], start=(k == 0),
                            stop=(k == K_CHUNKS - 1),
                        )
                    if k == 16:
                        # bias mid-stream (accumulation order is irrelevant):
                        # psum[m, n] += 1*b_hi[n] + 1*b_lo[n]
                        for j, (_, _, sz, oo) in enumerate(BANKS):
                            nc.tensor.matmul(
                                psums[j][:], lhsT=ones_t[:],
                                rhs=bias_t[:, oo:oo + sz], start=False, stop=False,
                            )
            # evictions split across DVE (bank0) and ACT (banks 1-2) into one
            # contiguous tile; out-DMA boundaries align with the writer split
            # so each DMA waits on exactly one engine. SWDGE queues are
            # otherwise unused, so the out-DMAs get a virgin queue (no
            # cross-trigger FIFO wait; walrus allows 1 wait per DMA).
            ob = obp.tile([P, N_SHARD], mybir.dt.float32)
            nc.vector.tensor_copy(out=ob[:, 0:512], in_=psums[0][:])
            nc.scalar.copy(out=ob[:, 512:COLS_A], in_=psums[1][:])
            nc.scalar.copy(out=ob[:, COLS_A:], in_=psums[2][:])
            nc.gpsimd.dma_start(out=out[:, :512], in_=ob[:, :512])
            nc.gpsimd.dma_start(out=out[:, 512:], in_=ob[:, 512:])
    return nc


def get_nc():
    if "nc" not in _CACHE:
        _CACHE["nc"] = _build_nc()
    return _CACHE["nc"]


def make_in_maps(x, w_q, scale, bias):
    """Host-side shard/layout prep. Returns list of 8 per-core input dicts."""
    x = np.asarray(x, dtype=np.float32).reshape(P, IN_F)
    s = float(np.asarray(scale).reshape(-1)[0])
    xs = (x * s).astype(np.float16)
    # SBUF layout: x_sb[p, nk*128+m] = xs[m, nk*128+p] (contraction on partitions)
    x_sb = np.ascontiguousarray(
        xs.reshape(P, K_CHUNKS, P).transpose(2, 1, 0)
    ).reshape(P, IN_F)

    w8 = np.asarray(w_q).astype(np.int8)
    wT = w8.T  # [IN_F, OUT_F]

    b32 = np.asarray(bias, dtype=np.float32)
    b_hi = b32.astype(np.float16)
    b_lo = (b32 - b_hi.astype(np.float32)).astype(np.float16)

    in_maps = []
    for c in range(N_CORES):
        lo, hi = c * N_SHARD, (c + 1) * N_SHARD
        shard = wT[:, lo:hi]
        in_maps.append({
            "xs": x_sb,
            "wq": np.ascontiguousarray(shard),
            "bias2": np.ascontiguousarray(
                np.stack([b_hi[lo:hi], b_lo[lo:hi]], axis=0)
            ),
        })
    return in_maps


def gather(results):
    """results: list of 8 dicts with 'out' [P, N_SHARD] f32 -> full output."""
    full = np.concatenate([np.asarray(r["out"]) for r in results], axis=1)
    return np.ascontiguousarray(full.reshape(4, 32, OUT_F).astype(np.float32))


def kernel(x, w_q, scale, bias):
    from concourse.bass_utils import run_bass_kernel_spmd

    nc = get_nc()
    in_maps = make_in_maps(x, w_q, scale, bias)
    res = run_bass_kernel_spmd(nc, in_maps, list(range(N_CORES)))
    return gather(res.results)

